# revision 28
# baseline (speedup 1.0000x reference)
"""MicroGPT forward pass on 8 Trainium2 NeuronCores (Bass/Tile), v2.

Sharding: token-sharded — core c = 2*b + h owns batch b, sequence half h
(512 contiguous tokens). Activations feature-major (x^T tiles [128, 512]).

v2 vs baseline (1.43ms):
- LayerNorm folded into the matmuls: gammas are 1 and betas 0 in this
  model, so W @ LN(x) = rstd_t * (W @ x) - (mean_t*rstd_t) * (W @ 1).
  Projections/fc1 run on raw x (cast f16) with per-token corrections on
  the vector engine afterwards — the serial LN barrier that idled the PE
  ~11.5us twice per layer is gone.
- Own/remote K split into separate tiles so own-slot attention no longer
  falsely depends on the AllGather's write into KT[:, T:2T].
- Softmax denominators + LN row stats broadcast across partitions with
  gpsimd.partition_broadcast (idle engine) instead of DRAM round trips.
- Gelu via the native Gelu_apprx_tanh activation (1 op, was 6).
- Final LN normalizes with gpsimd broadcasts; unembed chunks prefetch.
Attention: S^T = K^T-slice x Q^T; softmax without max subtraction;
denominators via ones column in V. K/V slots 0-3 own, 4-7 pair (remw
0/1 weights); per layer an 8-core AllGather shares K^T/V; final token:
masked AllReduce + vocab-sharded unembed.
"""
import sys, math

sys.path.insert(0, "/opt/trn_rl_repo")
import numpy as np

import concourse.bass as bass
import concourse.bacc as bacc
import concourse.mybir as mybir
import concourse.tile as tile

D, NH, DH, FF, NL, V = 768, 12, 64, 3072, 4, 32000
B, S = 4, 1024
EPS = 1e-5
NC_ = 8
P = 128
T = 512            # tokens per core
DT = D // P        # 6 d-model tiles
FT = FF // P       # 24 ff tiles
KB = 8             # key slots (0-3 own, 4-7 pair)
VS = V // NC_      # 4000 vocab rows per core
VCH = 8            # vocab chunks of 500
VCW = VS // VCH    # 500
F32 = mybir.dt.float32
F32R = mybir.dt.float32r
F16 = mybir.dt.float16
F8 = mybir.dt.float8e4
I32 = mybir.dt.int32
AF = mybir.ActivationFunctionType
OP = mybir.AluOpType
SCALE = 1.0 / math.sqrt(DH)
VW = NH * (DH + 1)           # 780 — V tile width incl. ones cols
CONTRIB_W = DT * T + 4 * VW  # AllGather contribution width
NW1 = 2 * DT + FT            # 36 packed w1 columns (K, Q, fc1)


# ---------------------------------------------------------------- bass program
def build_nc(n_layers=NL, pcol=511, dbg=False):
    nc = bacc.Bacc(None, target_bir_lowering=False, debug=False, num_devices=NC_)

    x0T = nc.dram_tensor("x0T", [DT, P, T], F32R, kind="ExternalInput")
    wqT = nc.dram_tensor("wqT", [n_layers, P, DT * D], F16, kind="ExternalInput")
    wkT = nc.dram_tensor("wkT", [n_layers, P, DT * D], F16, kind="ExternalInput")
    wvT = nc.dram_tensor("wvT", [n_layers, P, DT * D], F16, kind="ExternalInput")
    woT = nc.dram_tensor("woT", [n_layers, P, DT * D], F16, kind="ExternalInput")
    fc1T = nc.dram_tensor("fc1T", [n_layers, FT, P, DT * P], F16, kind="ExternalInput")
    fc2T = nc.dram_tensor("fc2T", [n_layers, FT, P, D], F16, kind="ExternalInput")
    w1r = nc.dram_tensor("w1r", [n_layers, 1, 2 * D + FF], F16, kind="ExternalInput")
    w1v = nc.dram_tensor("w1v", [n_layers, 1, D], F16, kind="ExternalInput")
    uT = nc.dram_tensor("uT", [DT, P, VS], F16, kind="ExternalInput")
    u1d = nc.dram_tensor("u1d", [1, VS], F16, kind="ExternalInput")
    masks = nc.dram_tensor("masks", [P, P], F16, kind="ExternalInput")
    remw = nc.dram_tensor("remw", [P, 1], F32, kind="ExternalInput")
    sel4 = nc.dram_tensor("sel4", [P, B], F32R, kind="ExternalInput")
    pairsel = nc.dram_tensor("pairsel", [1, 1], I32, kind="ExternalInput")

    out = nc.dram_tensor("out", [B, VS], F32, kind="ExternalOutput")
    if dbg:
        dbgx = nc.dram_tensor("dbgx", [DT, P, T], F32, kind="ExternalOutput")

    from contextlib import ExitStack
    with tile.TileContext(nc) as tc:
        with ExitStack() as stk:
            cpool = stk.enter_context(tc.tile_pool(name="const", bufs=1))
            ppool = stk.enter_context(tc.tile_pool(name="persist", bufs=1))
            xpool = stk.enter_context(tc.tile_pool(name="xp", bufs=6))
            hpool = stk.enter_context(tc.tile_pool(name="hp", bufs=7))
            qpool = stk.enter_context(tc.tile_pool(name="qp", bufs=6))
            apool = stk.enter_context(tc.tile_pool(name="ac", bufs=6))
            wpool = stk.enter_context(tc.tile_pool(name="wp", bufs=8))
            fpool = stk.enter_context(tc.tile_pool(name="fp", bufs=4))
            epool = stk.enter_context(tc.tile_pool(name="ep", bufs=6))
            spool = stk.enter_context(tc.tile_pool(name="sp", bufs=5))
            bpool = stk.enter_context(tc.tile_pool(name="bp", bufs=2))
            npool = stk.enter_context(tc.tile_pool(name="np_", bufs=3))
            espool = stk.enter_context(tc.tile_pool(name="es", bufs=12))
            x8pool = stk.enter_context(tc.tile_pool(name="x8", bufs=1))
            psm = stk.enter_context(tc.tile_pool(name="psm", bufs=6, space="PSUM"))
            psr = stk.enter_context(tc.tile_pool(name="psr", bufs=2, space="PSUM"))
            dpool = stk.enter_context(tc.tile_pool(name="dram", bufs=2, space="DRAM"))
            # ---- constants (memset cannot write f32r; stage via f32 + copy)
            ones_f32 = cpool.tile([P, 1], F32)
            nc.vector.memset(ones_f32[:], 1.0)
            ones_f16 = cpool.tile([P, 1], F16)
            nc.vector.memset(ones_f16[:], 1.0)
            trimask = cpool.tile([P, P], F16)
            nc.sync.dma_start(trimask[:], masks[:])
            ones_col = cpool.tile([P, 1], F32R)
            nc.vector.tensor_copy(ones_col[:], ones_f32[:])
            eps1 = cpool.tile([1, 1], F32)
            nc.vector.memset(eps1[:], EPS)
            sel4_sb = cpool.tile([P, B], F32R)
            nc.sync.dma_start(sel4_sb[:], sel4[:])
            remw_sb = cpool.tile([P, 1], F32)
            nc.sync.dma_start(remw_sb[:], remw[:])

            # persistent K^T / V buffers
            KTo = [ppool.tile([P, T], F16, tag=f"kto{e}", name=f"KTo{e}")
                   for e in range(DT)]
            KTr = [ppool.tile([P, T], F16, tag=f"ktr{e}", name=f"KTr{e}")
                   for e in range(DT)]
            VT = [ppool.tile([P, VW], F16, tag=f"vt{j}", name=f"VT{j}")
                  for j in range(KB)]
            for j in range(4):
                for h in range(NH):
                    nc.vector.tensor_copy(
                        VT[j][:, h * (DH + 1) + DH : h * (DH + 1) + DH + 1],
                        ones_f32[:])

            # pair rank register for dynamic reads of the AllGather output
            with tc.tile_critical():
                with nc.sync.register("pairreg") as preg:
                    nc.sync.reg_load(preg, pairsel[0:1, 0:1])
                    pv = nc.sync.snap(preg, min_val=0, max_val=NC_ - 1)

            # ---- residual stream (updated in place by residual adds)
            xT = []
            for k in range(DT):
                t_ = xpool.tile([P, T], F32R, tag="xT", name=f"xT{k}")
                nc.sync.dma_start(t_[:], x0T[k])
                xT.append(t_)

            def ln_stats(w1_sb, lname):
                """Fold-LN prep: returns (xh f16 tiles, rstd_b, mrs_b,
                negmean_h row, rstd_row) — no barrier on the PE."""
                sum_ps = psm.tile([1, T], F32, tag="acc", space="PSUM",
                                  name=f"{lname}sum")
                sq_ps = psm.tile([1, T], F32, tag="acc", space="PSUM",
                                 name=f"{lname}sq")
                xh = []
                for k in range(DT):
                    h_ = hpool.tile([P, T], F16, tag="hT", name=f"{lname}xh{k}")
                    nc.scalar.copy(h_[:], xT[k][:])
                    xsq = epool.tile([P, T], F16, tag="lnsq", name="xsq", bufs=3)
                    nc.vector.tensor_mul(xsq[:], h_[:], h_[:])
                    nc.tensor.matmul(sum_ps[:], ones_f16[:], h_[:],
                                     start=(k == 0), stop=(k == DT - 1))
                    nc.tensor.matmul(sq_ps[:], ones_f16[:], xsq[:],
                                     start=(k == 0), stop=(k == DT - 1))
                    xh.append(h_)
                sums_sb = spool.tile([1, T], F32, tag="lnstat", name="sums_sb")
                nc.vector.tensor_copy(sums_sb[:], sum_ps[:])
                m2s = spool.tile([1, T], F32, tag="lnstat", name="m2s")
                nc.vector.scalar_tensor_tensor(out=m2s[:], in0=sums_sb[:],
                                               scalar=1.0 / (D * D), in1=sums_sb[:],
                                               op0=OP.mult, op1=OP.mult)
                var = spool.tile([1, T], F32, tag="lnstat", name="var")
                nc.vector.scalar_tensor_tensor(out=var[:], in0=sq_ps[:],
                                               scalar=1.0 / D, in1=m2s[:],
                                               op0=OP.mult, op1=OP.subtract)
                std = spool.tile([1, T], F32, tag="lnstat", name="std")
                nc.scalar.activation(std[:], var[:], AF.Sqrt, bias=eps1[:])
                rstd = spool.tile([1, T], F32, tag="lnr", name="rstd", bufs=4)
                nc.vector.reciprocal_approx_fast(out=rstd[:], in_=std[:])
                negmean_h = spool.tile([1, T], F16, tag="lnr", name="negmean_h", bufs=4)
                nc.vector.tensor_scalar_mul(negmean_h[:], sums_sb[:], -1.0 / D)
                rstd_b = bpool.tile([P, T], F32, tag="lnb", name="rstd_b")
                nc.gpsimd.partition_broadcast(rstd_b[:], rstd[:])
                return xh, rstd_b, negmean_h, rstd

            for l in range(n_layers):
                with nc.named_scope(f"L{l}"):
                    # per-layer small constants (w1 row sums, f16 rows)
                    w1_sb = spool.tile([1, 2 * D + FF], F16, tag="w1", name="w1_sb", bufs=2)
                    nc.sync.dma_start(w1_sb[:], w1r[l])
                    w1v_sb = spool.tile([1, D], F16, tag="w1v", name="w1v_sb", bufs=2)
                    nc.sync.dma_start(w1v_sb[:], w1v[l])

                    xh, rstd_b, negmean_h, rstd = ln_stats(w1_sb, "ln1")
                    # rstd by-partition for the V write (token-major)
                    rmd = dpool.tile([1, T], F32, tag="rmd", name="rmd")
                    nc.sync.dma_start(rmd[:], rstd[:])
                    rstdT = spool.tile([P, 4], F32, tag="rstdT", name="rstdT")
                    nc.sync.dma_start(
                        rstdT[:],
                        rmd[0:1, :].rearrange("o (m p) -> (o p) m", p=P))

                    # ---- K^T, V first (feeds AllGather early), then Q^T
                    wk_sb = []
                    for k in range(DT):
                        wt = wpool.tile([P, D], F16, tag="w", name="wk_sb")
                        nc.sync.dma_start(wt[:], wkT[l][:, k * D : (k + 1) * D])
                        wk_sb.append(wt)
                    ko8 = []
                    for m in range(DT):
                        k_ps = psr.tile([P, T], F32, tag="rot", space="PSUM",
                                        name="k_ps")
                        for k in range(DT):
                            nc.tensor.matmul(
                                k_ps[:], wk_sb[k][:, m * P : (m + 1) * P],
                                xh[k][:], start=(k == 0), stop=False)
                        nc.tensor.matmul(
                            k_ps[:], w1_sb[0:1, m * P : (m + 1) * P],
                            negmean_h[:], start=False, stop=True)
                        nc.vector.tensor_mul(KTo[m][:], k_ps[:], rstd_b[:])
                        k8 = x8pool.tile([P, T], F8, tag=f"ko8{m}", name="k8")
                        nc.vector.tensor_copy(k8[:], KTo[m][:])
                        ko8.append(k8)

                    wv_sb = []
                    for k in range(DT):
                        wt = wpool.tile([P, D], F16, tag="w", name="wv_sb")
                        nc.sync.dma_start(wt[:], wvT[l][:, k * D : (k + 1) * D])
                        wv_sb.append(wt)
                    vo8 = []
                    for m in range(4):
                        for c in range(2):
                            v_ps = psr.tile([P, 6 * DH], F32, tag="rot",
                                            space="PSUM", name="v_ps")
                            for k in range(DT):
                                nc.tensor.matmul(
                                    v_ps[:], xh[k][:, m * P : (m + 1) * P],
                                    wv_sb[k][:, c * 6 * DH : (c + 1) * 6 * DH],
                                    start=(k == 0), stop=False)
                            # rank-1: -= mean_t * w1v  (negmean x w1v outer)
                            nc.tensor.matmul(
                                v_ps[:], negmean_h[0:1, m * P : (m + 1) * P],
                                w1v_sb[0:1, c * 6 * DH : (c + 1) * 6 * DH],
                                start=False, stop=True)
                            dst = VT[m][:, c * 6 * (DH + 1) : (c + 1) * 6 * (DH + 1)] \
                                .rearrange("p (h e) -> p h e", h=6, e=DH + 1)[:, :, 0:DH]
                            src = v_ps[:].rearrange("p (h e) -> p h e", h=6, e=DH)
                            nc.scalar.activation(dst, src, AF.Identity,
                                                 scale=rstdT[:, m : m + 1])
                        v8 = x8pool.tile([P, VW], F8, tag=f"vo8{m}", name="v8")
                        nc.vector.tensor_copy(v8[:], VT[m][:])
                        vo8.append(v8)

                    # ---- share K^T/V with the pair core (8-way fp8 AllGather)
                    contrib = dpool.tile([P, CONTRIB_W], F8, tag="contrib",
                                         name="contrib")
                    for e in range(DT):
                        nc.sync.dma_start(contrib[:, e * T : (e + 1) * T], ko8[e][:])
                    for m in range(4):
                        nc.sync.dma_start(
                            contrib[:, DT * T + m * VW : DT * T + (m + 1) * VW],
                            vo8[m][:])
                    gout = dpool.tile([NC_, P, CONTRIB_W], F8, tag="gout",
                                      addr_space="Shared", name="gout")
                    nc.gpsimd.collective_compute(
                        "AllGather", OP.bypass,
                        ins=[contrib[:].opt()],
                        outs=[gout[:].opt()],
                        replica_groups=[list(range(NC_))],
                    )

                    # ---- pre-AG-independent work: Q^T + own-slot scores
                    wq_sb = []
                    for k in range(DT):
                        wt = wpool.tile([P, D], F16, tag="w", name="wq_sb")
                        nc.sync.dma_start(wt[:], wqT[l][:, k * D : (k + 1) * D])
                        wq_sb.append(wt)
                    QT = []
                    for m in range(DT):
                        q_ps = psr.tile([P, T], F32, tag="rot", space="PSUM",
                                        name="q_ps")
                        for k in range(DT):
                            nc.tensor.matmul(
                                q_ps[:], wq_sb[k][:, m * P : (m + 1) * P],
                                xh[k][:], start=(k == 0), stop=False)
                        nc.tensor.matmul(
                            q_ps[:], w1_sb[0:1, D + m * P : D + (m + 1) * P],
                            negmean_h[:], start=False, stop=True)
                        qt = qpool.tile([P, T], F16, tag="qt", name="qt")
                        nc.vector.tensor_mul(qt[:], q_ps[:], rstd_b[:])
                        QT.append(qt)

                    # phase A: scores+exp+mask for own slots, all 12 heads
                    # (es saved in SBUF; everything here is AG-independent)
                    es_own = {}
                    for h in range(NH):
                        et, base = h // 2, (h % 2) * DH
                        for j in range(4):
                            c0 = j * P
                            N = T - c0
                            pp = psr if h % 2 == 0 else psm
                            s_ps = pp.tile([P, T], F32,
                                           tag="rot" if h % 2 == 0 else "acc",
                                           space="PSUM", name="s_ps")
                            nc.tensor.matmul(
                                s_ps[:, 0:N],
                                KTo[et][base : base + DH, c0 : c0 + P],
                                QT[et][base : base + DH, c0:T],
                                start=True, stop=True)
                            e_sb = espool.tile([P, N], F16, tag=f"esv{j}",
                                               name="e_sb")
                            nc.scalar.activation(e_sb[:], s_ps[:, 0:N],
                                                 AF.Exp, scale=SCALE)
                            nc.vector.tensor_mul(e_sb[:, 0:P], e_sb[:, 0:P],
                                                 trimask[:])
                            es_own[(h, j)] = e_sb

                    # ---- AG-dependent: unpack pair K/V (fp8 -> f16, V *remw)
                    rsrc = gout[bass.ds(pv, 1)]
                    for e in range(DT):
                        kr8 = x8pool.tile([P, T], F8, tag=f"kr8{e}", name="kr8")
                        nc.sync.dma_start(kr8[:], rsrc[0, :, e * T : (e + 1) * T])
                        nc.vector.tensor_copy(KTr[e][:], kr8[:])
                    for m in range(4):
                        vr8 = x8pool.tile([P, VW], F8, tag=f"vr8{m}", name="vr8")
                        nc.sync.dma_start(
                            vr8[:],
                            rsrc[0, :, DT * T + m * VW : DT * T + (m + 1) * VW])
                        nc.vector.tensor_scalar_mul(VT[4 + m][:], vr8[:],
                                                    remw_sb[:, 0:1])

                    # phase B: attnV own (saved es) + remote slots + normalize
                    wo_sb = []
                    for k in range(DT):
                        wt = wpool.tile([P, D], F16, tag="w", name="wo_sb")
                        nc.sync.dma_start(wt[:], woT[l][:, k * D : (k + 1) * D])
                        wo_sb.append(wt)
                    attnC = [apool.tile([P, T], F16, tag="attnC", name=f"attnC{e}")
                             for e in range(DT)]
                    for hg in range(0, NH, 4):
                        attn_ps = {}
                        for h in range(hg, hg + 4):
                            attn_ps[h] = psm.tile([DH + 1, T], F32, tag="acc",
                                                  space="PSUM", name=f"attnps{h}")
                        for j in range(4):
                            c0 = j * P
                            for h in range(hg, hg + 4):
                                nc.tensor.matmul(
                                    attn_ps[h][:, c0:T],
                                    VT[j][:, h * (DH + 1) : (h + 1) * (DH + 1)],
                                    es_own[(h, j)][:],
                                    start=(j == 0), stop=False)
                        for j in range(4, KB):
                            es = {}
                            for h in range(hg, hg + 4):
                                et, base = h // 2, (h % 2) * DH
                                pp = psr if h % 2 == 0 else psm
                                s_ps = pp.tile([P, T], F32,
                                               tag="rot" if h % 2 == 0 else "acc",
                                               space="PSUM", name="s_ps")
                                nc.tensor.matmul(
                                    s_ps[:],
                                    KTr[et][base : base + DH, (j - 4) * P : (j - 3) * P],
                                    QT[et][base : base + DH, :],
                                    start=True, stop=True)
                                e_sb = epool.tile([P, T], F16, tag="e", name="e_sb")
                                nc.scalar.activation(e_sb[:], s_ps[:],
                                                     AF.Exp, scale=SCALE)
                                es[h] = e_sb
                            for h in range(hg, hg + 4):
                                nc.tensor.matmul(
                                    attn_ps[h][:],
                                    VT[j][:, h * (DH + 1) : (h + 1) * (DH + 1)],
                                    es[h][:],
                                    start=False, stop=(j == KB - 1))
                        for h in range(hg, hg + 4):
                            den = spool.tile([1, T], F32, tag="recip", name="den", bufs=3)
                            nc.vector.tensor_copy(den[:], attn_ps[h][DH : DH + 1, :])
                            recip = spool.tile([1, T], F32, tag="recip", name="recip", bufs=3)
                            nc.vector.reciprocal_approx_fast(out=recip[:], in_=den[:])
                            nrm_b = npool.tile([DH, T], F32, tag="nrm", name="nrm_b")
                            nc.gpsimd.partition_broadcast(nrm_b[:], recip[:])
                            base = (h % 2) * DH
                            nc.vector.tensor_mul(
                                attnC[h // 2][base : base + DH, :],
                                attn_ps[h][0:DH, :], nrm_b[:])
                        # wo partial for this hg's two attnC tiles; fills the
                        # PE while the next hg waits on exp/normalize
                        for m in range(DT):
                            o_ps = psr.tile([P, T], F32, tag="rot", space="PSUM",
                                            name="o_ps")
                            for kk in range(2):
                                k = 2 * (hg // 4) + kk
                                nc.tensor.matmul(
                                    o_ps[:],
                                    wo_sb[k][:, m * P : (m + 1) * P],
                                    attnC[k][:], start=(kk == 0), stop=(kk == 1))
                            nc.vector.tensor_add(xT[m][:], o_ps[:], xT[m][:])

                    # ---- FFN (fold-LN + native gelu)
                    xh2, rstd2_b, negmean2_h, _r2 = ln_stats(w1_sb, "ln2")
                    x2_ps = [psm.tile([P, T], F32, tag="acc", space="PSUM",
                                      name=f"x2ps{m}") for m in range(DT)]
                    for f in range(FT):
                        f1w = fpool.tile([P, DT * P], F16, tag="f1w", name="f1w")
                        nc.sync.dma_start(f1w[:], fc1T[l, f])
                        f1_ps = psr.tile([P, T], F32, tag="rot", space="PSUM",
                                         name="f1_ps")
                        for k in range(DT):
                            nc.tensor.matmul(f1_ps[:], f1w[:, k * P : (k + 1) * P],
                                             xh2[k][:], start=(k == 0),
                                             stop=False)
                        nc.tensor.matmul(
                            f1_ps[:], w1_sb[0:1, 2 * D + f * P : 2 * D + (f + 1) * P],
                            negmean2_h[:], start=False, stop=True)
                        t2 = epool.tile([P, T], F32, tag="lntmp", name="f1fix", bufs=4)
                        nc.vector.tensor_mul(t2[:], f1_ps[:], rstd2_b[:])
                        g_sb = epool.tile([P, T], F16, tag="e", name="g_sb")
                        nc.scalar.activation(g_sb[:], t2[:], AF.Gelu_apprx_tanh)
                        f2w = fpool.tile([P, D], F16, tag="f2w", name="f2w")
                        nc.sync.dma_start(f2w[:], fc2T[l, f])
                        for m in range(DT):
                            nc.tensor.matmul(x2_ps[m][:], f2w[:, m * P : (m + 1) * P],
                                             g_sb[:], start=(f == 0),
                                             stop=(f == FT - 1))
                    for m in range(DT):
                        nc.vector.tensor_add(xT[m][:], x2_ps[m][:], xT[m][:])

            # ---- final: masked AllReduce of predicted token's x column
            with nc.named_scope("final"):
                if dbg:
                    for k in range(DT):
                        nc.sync.dma_start(dbgx[k], xT[k][:].bitcast(F32))
                cont = dpool.tile([P, DT * B], F32, tag="cont", name="cont")
                csb = spool.tile([P, DT * B], F32, tag="csb", name="csb", bufs=1)
                for k in range(DT):
                    nc.vector.tensor_mul(
                        csb[:, k * B : (k + 1) * B],
                        xT[k][:, pcol : pcol + 1].to_broadcast((P, B)),
                        sel4_sb[:])
                nc.sync.dma_start(cont[:], csb[:])
                ar_out = dpool.tile([P, DT * B], F32, tag="arout",
                                    addr_space="Shared", name="ar_out")
                nc.gpsimd.collective_compute(
                    "AllReduce", OP.add,
                    ins=[cont[:].opt()],
                    outs=[ar_out[:].opt()],
                    replica_groups=[list(range(NC_))],
                )
                xf_raw = spool.tile([P, DT * B], F32, tag="xfraw", name="xf_raw", bufs=1)
                nc.sync.dma_start(xf_raw[:], ar_out[:])
                xf = spool.tile([P, DT * B], F32R, tag="xf", name="xf", bufs=1)
                nc.vector.tensor_copy(xf[:], xf_raw[:])

                fs_ps = psm.tile([1, B], F32, tag="acc", space="PSUM", name="fs_ps")
                fq_ps = psm.tile([1, B], F32, tag="acc", space="PSUM", name="fq_ps")
                xfsq = spool.tile([P, DT * B], F32R, tag="xfsq", name="xfsq", bufs=1)
                nc.vector.tensor_mul(xfsq[:], xf[:], xf[:])
                for k in range(DT):
                    nc.tensor.matmul(fs_ps[:], ones_col[:], xf[:, k * B : (k + 1) * B],
                                     start=(k == 0), stop=(k == DT - 1))
                    nc.tensor.matmul(fq_ps[:], ones_col[:], xfsq[:, k * B : (k + 1) * B],
                                     start=(k == 0), stop=(k == DT - 1))
                fmean = spool.tile([1, B], F32, tag="lnstat", name="fmean")
                nc.vector.tensor_scalar_mul(fmean[:], fs_ps[:], 1.0 / D)
                fm2 = spool.tile([1, B], F32, tag="lnstat", name="fm2")
                nc.vector.tensor_mul(fm2[:], fmean[:], fmean[:])
                fsqd = spool.tile([1, B], F32, tag="lnstat", name="fsqd")
                nc.vector.tensor_scalar_mul(fsqd[:], fq_ps[:], 1.0 / D)
                fvar = spool.tile([1, B], F32, tag="lnstat", name="fvar")
                nc.vector.tensor_sub(fvar[:], fsqd[:], fm2[:])
                fstd = spool.tile([1, B], F32, tag="lnstat", name="fstd")
                nc.scalar.activation(fstd[:], fvar[:], AF.Sqrt, bias=eps1[:])
                frstd = spool.tile([1, B], F32, tag="lnr", name="frstd", bufs=4)
                nc.vector.reciprocal(frstd[:], fstd[:])
                fmrsn = spool.tile([1, B], F32, tag="lnr", name="fmrsn", bufs=4)
                nc.vector.scalar_tensor_tensor(out=fmrsn[:], in0=fmean[:],
                                               scalar=-1.0, in1=frstd[:],
                                               op0=OP.mult, op1=OP.mult)
                # transpose [frstd | -mean*rstd] rows to per-partition columns
                fpack = spool.tile([1, 2 * B], F32, tag="lnstat", name="fpack")
                nc.vector.tensor_copy(fpack[:, 0:B], frstd[:])
                nc.vector.tensor_copy(fpack[:, B : 2 * B], fmrsn[:])
                frd = dpool.tile([1, 2 * B], F32, tag="frd", name="frd")
                nc.sync.dma_start(frd[:], fpack[:])
                frstd_c = spool.tile([B, 1], F32, tag="lnr", name="frstd_c", bufs=4)
                nc.sync.dma_start(frstd_c[:], frd[0:1, 0:B].rearrange("o a -> a o"))
                fmrsn_c = spool.tile([B, 1], F32, tag="lnr", name="fmrsn_c", bufs=4)
                nc.sync.dma_start(fmrsn_c[:],
                                  frd[0:1, B : 2 * B].rearrange("o a -> a o"))
                # unembed on RAW xf (LN folded into a per-batch correction)
                xfn = spool.tile([P, DT * B], F16, tag="xfn", name="xfn", bufs=1)
                nc.scalar.copy(xfn[:], xf[:])
                u1b = spool.tile([B, VS], F16, tag="u1b", name="u1b", bufs=1)
                nc.sync.dma_start(u1b[:], u1d[0:1, :].partition_broadcast(B).opt())

                for ci in range(VCH):
                    lg_ps = psr.tile([B, VCW], F32, tag="rot", space="PSUM",
                                     name="lg_ps")
                    for k in range(DT):
                        u_sb = qpool.tile([P, VCW], F16, tag="qt", name="u_sb")
                        nc.sync.dma_start(u_sb[:], uT[k, :, ci * VCW : (ci + 1) * VCW])
                        nc.tensor.matmul(lg_ps[:], xfn[:, k * B : (k + 1) * B], u_sb[:],
                                         start=(k == 0), stop=(k == DT - 1))
                    tsc = fpool.tile([B, VCW], F32, tag="och", name="tsc", bufs=2)
                    nc.scalar.activation(tsc[:], lg_ps[:], AF.Identity,
                                         scale=frstd_c[:])
                    och = fpool.tile([B, VCW], F32, tag="och", name="och", bufs=2)
                    nc.vector.scalar_tensor_tensor(
                        out=och[:], in0=u1b[:, ci * VCW : (ci + 1) * VCW],
                        scalar=fmrsn_c[:], in1=tsc[:],
                        op0=OP.mult, op1=OP.add)
                    nc.sync.dma_start(out[:, ci * VCW : (ci + 1) * VCW], och[:])

    nc.compile()
    return nc


# ---------------------------------------------------------------- host side
def _positional_encoding(s, d):
    idx = np.arange(d)
    exponent = ((2 * (idx // 2)).astype(np.float32) / float(d)).astype(np.float32)
    pos = np.arange(s, dtype=np.float32)[:, None]
    angle = pos / np.power(np.float32(10000.0), exponent[None, :], dtype=np.float32)
    return np.where((idx % 2 == 0)[None, :], np.sin(angle), np.cos(angle)).astype(np.float32)


def _build_masks():
    """trimask[r, c] = 1 if key r <= query c (within-block causal)."""
    r = np.arange(P)
    return (r[:, None] <= r[None, :]).astype(np.float16)


def prepare_inputs(tokens, predict_idx, embedding, ln1_g, ln1_b, wq, wk, wv, wo,
                   ln2_g, ln2_b, fc1, fc2, lnf_g, lnf_b, unembed, n_layers=NL):
    f = lambda a: np.ascontiguousarray(np.asarray(a), dtype=np.float32)
    # the fold-LN kernel exploits gamma=1 / beta=0 (true for this model)
    for g in (ln1_g, ln2_g, lnf_g):
        assert np.allclose(np.asarray(g), 1.0), "LN gamma must be 1"
    for b in (ln1_b, ln2_b, lnf_b):
        assert np.allclose(np.asarray(b), 0.0), "LN beta must be 0"
    tokens = np.asarray(tokens)
    emb = f(embedding)
    pos = _positional_encoding(S, D)

    def wlayout(a):  # [L, out, in] -> [L, P, DT*D] with [l, p, k*D + dout]
        aT = a.transpose(0, 2, 1)
        return np.ascontiguousarray(
            aT.reshape(n_layers, DT, P, D).transpose(0, 2, 1, 3)
            .reshape(n_layers, P, DT * D)).astype(np.float16)

    wqf = f(wq)[:n_layers].reshape(-1, NH * DH, D)
    wkf = f(wk)[:n_layers].reshape(-1, NH * DH, D)
    wvf = f(wv)[:n_layers].reshape(-1, NH * DH, D)
    wqT = wlayout(wqf)
    wkT = wlayout(wkf)
    wvT = wlayout(wvf)
    woT = wlayout(f(wo)[:n_layers])
    fc1f = f(fc1)[:n_layers]
    fc1T = np.ascontiguousarray(
        fc1f.transpose(0, 2, 1)
        .reshape(n_layers, DT, P, FT, P).transpose(0, 3, 2, 1, 4)
        .reshape(n_layers, FT, P, DT * P)).astype(np.float16)
    fc2T = np.ascontiguousarray(
        f(fc2)[:n_layers].transpose(0, 2, 1)
        .reshape(n_layers, FT, P, D)).astype(np.float16)
    uTf = np.ascontiguousarray(f(unembed).T.reshape(DT, P, V)).astype(np.float16)
    u1f = np.ascontiguousarray(
        f(unembed).astype(np.float16).astype(np.float32).sum(-1)
        .reshape(1, V)).astype(np.float16)

    # packed row sums [L, 1, 768(K) + 768(Q) + 3072(fc1)] for the rank-1
    # -mean*w1 correction matmuls (rhs = negmean row)
    w1r_ = np.ascontiguousarray(np.concatenate(
        [wkf.sum(-1), wqf.sum(-1), fc1f.sum(-1)],
        axis=1).reshape(n_layers, 1, 2 * D + FF)).astype(np.float16)
    w1v_ = np.ascontiguousarray(wvf.sum(-1).reshape(n_layers, 1, D)).astype(np.float16)

    masks = _build_masks()
    pidx = int(predict_idx)
    in_maps = []
    for c in range(NC_):
        b, h = c // 2, c % 2
        toks = np.asarray(tokens[b, h * T : (h + 1) * T]).astype(np.int64)
        x0 = emb.T[toks] + pos[h * T : (h + 1) * T]
        x0T = np.ascontiguousarray(x0.T.reshape(DT, P, T)).astype(np.float32)
        sel4 = np.zeros((P, B), np.float32)
        if pidx // T == h:
            sel4[:, b] = 1.0
        m = {
            "x0T": x0T, "wqT": wqT, "wkT": wkT, "wvT": wvT, "woT": woT,
            "fc1T": fc1T, "fc2T": fc2T, "w1r": w1r_, "w1v": w1v_,
            "uT": uTf[:, :, c * VS : (c + 1) * VS].copy(),
            "u1d": u1f[:, c * VS : (c + 1) * VS].copy(),
            "masks": masks,
            "remw": np.full((P, 1), 1.0 if h == 1 else 0.0, np.float32),
            "sel4": sel4,
            "pairsel": np.array([[c ^ 1]], np.int32),
        }
        in_maps.append(m)
    return in_maps


_CACHED = {}


def kernel(**inputs):
    from concourse.bass_utils import run_bass_kernel_spmd
    pidx = int(np.asarray(inputs["predict_idx"]))
    key = ("nc", pidx % T)
    if key not in _CACHED:
        _CACHED[key] = build_nc(pcol=pidx % T)
    nc = _CACHED[key]
    in_maps = prepare_inputs(**inputs)
    res = run_bass_kernel_spmd(nc, in_maps, core_ids=list(range(NC_)), trace=False)
    return np.concatenate([res.results[c]["out"] for c in range(NC_)], axis=1)


# revision 32
# speedup vs baseline: 1.0868x; 1.0868x over previous
"""MicroGPT forward pass on 8 Trainium2 NeuronCores (Bass/Tile), v2.

Sharding: token-sharded — core c = 2*b + h owns batch b, sequence half h
(512 contiguous tokens). Activations feature-major (x^T tiles [128, 512]).

v2 vs baseline (1.43ms):
- LayerNorm folded into the matmuls: gammas are 1 and betas 0 in this
  model, so W @ LN(x) = rstd_t * (W @ x) - (mean_t*rstd_t) * (W @ 1).
  Projections/fc1 run on raw x (cast f16) with per-token corrections on
  the vector engine afterwards — the serial LN barrier that idled the PE
  ~11.5us twice per layer is gone.
- Own/remote K split into separate tiles so own-slot attention no longer
  falsely depends on the AllGather's write into KT[:, T:2T].
- Softmax denominators + LN row stats broadcast across partitions with
  gpsimd.partition_broadcast (idle engine) instead of DRAM round trips.
- Gelu via the native Gelu_apprx_tanh activation (1 op, was 6).
- Final LN normalizes with gpsimd broadcasts; unembed chunks prefetch.
Attention: S^T = K^T-slice x Q^T; softmax without max subtraction;
denominators via ones column in V. K/V slots 0-3 own, 4-7 pair (remw
0/1 weights); per layer an 8-core AllGather shares K^T/V; final token:
masked AllReduce + vocab-sharded unembed.
"""
import sys, math

sys.path.insert(0, "/opt/trn_rl_repo")
import numpy as np

import concourse.bass as bass
import concourse.bacc as bacc
import concourse.mybir as mybir
import concourse.tile as tile

D, NH, DH, FF, NL, V = 768, 12, 64, 3072, 4, 32000
B, S = 4, 1024
EPS = 1e-5
NC_ = 8
P = 128
T = 512            # tokens per core
DT = D // P        # 6 d-model tiles
FT = FF // P       # 24 ff tiles
KB = 8             # key slots (0-3 own, 4-7 pair)
VS = V // NC_      # 4000 vocab rows per core
VCH = 8            # vocab chunks of 500
VCW = VS // VCH    # 500
F32 = mybir.dt.float32
F32R = mybir.dt.float32r
F16 = mybir.dt.float16
F8 = mybir.dt.float8e4
I32 = mybir.dt.int32
AF = mybir.ActivationFunctionType
OP = mybir.AluOpType
SCALE = 1.0 / math.sqrt(DH)
VW = NH * (DH + 1)           # 780 — V tile width incl. ones cols
CONTRIB_W = DT * T + 4 * VW  # AllGather contribution width
NW1 = 2 * DT + FT            # 36 packed w1 columns (K, Q, fc1)


# ---------------------------------------------------------------- bass program
def build_nc(n_layers=NL, pcol=511, dbg=False):
    nc = bacc.Bacc(None, target_bir_lowering=False, debug=False, num_devices=NC_)

    x0T = nc.dram_tensor("x0T", [DT, P, T], F32R, kind="ExternalInput")
    wqT = nc.dram_tensor("wqT", [n_layers, P, DT * D], F16, kind="ExternalInput")
    wkT = nc.dram_tensor("wkT", [n_layers, P, DT * D], F16, kind="ExternalInput")
    wvT = nc.dram_tensor("wvT", [n_layers, P, DT * D], F16, kind="ExternalInput")
    woT = nc.dram_tensor("woT", [n_layers, P, DT * D], F16, kind="ExternalInput")
    fc1T = nc.dram_tensor("fc1T", [n_layers, FT, P, DT * P], F16, kind="ExternalInput")
    fc2T = nc.dram_tensor("fc2T", [n_layers, FT, P, D], F16, kind="ExternalInput")
    w1r = nc.dram_tensor("w1r", [n_layers, 1, 2 * D + FF], F16, kind="ExternalInput")
    w1v = nc.dram_tensor("w1v", [n_layers, 1, D], F16, kind="ExternalInput")
    uT = nc.dram_tensor("uT", [DT, P, VS], F16, kind="ExternalInput")
    u1d = nc.dram_tensor("u1d", [1, VS], F16, kind="ExternalInput")
    masks = nc.dram_tensor("masks", [P, P], F16, kind="ExternalInput")
    remw = nc.dram_tensor("remw", [P, 1], F32, kind="ExternalInput")
    sel4 = nc.dram_tensor("sel4", [P, B], F32R, kind="ExternalInput")
    pairsel = nc.dram_tensor("pairsel", [1, 1], I32, kind="ExternalInput")

    out = nc.dram_tensor("out", [B, VS], F32, kind="ExternalOutput")
    if dbg:
        dbgx = nc.dram_tensor("dbgx", [DT, P, T], F32, kind="ExternalOutput")

    from contextlib import ExitStack
    with tile.TileContext(nc) as tc:
        with ExitStack() as stk:
            cpool = stk.enter_context(tc.tile_pool(name="const", bufs=1))
            ppool = stk.enter_context(tc.tile_pool(name="persist", bufs=1))
            xpool = stk.enter_context(tc.tile_pool(name="xp", bufs=6))
            hpool = stk.enter_context(tc.tile_pool(name="hp", bufs=6))
            qpool = stk.enter_context(tc.tile_pool(name="qp", bufs=6))
            apool = stk.enter_context(tc.tile_pool(name="ac", bufs=6))
            wpool = stk.enter_context(tc.tile_pool(name="wp", bufs=12))
            fpool = stk.enter_context(tc.tile_pool(name="fp", bufs=4))
            epool = stk.enter_context(tc.tile_pool(name="ep", bufs=6))
            spool = stk.enter_context(tc.tile_pool(name="sp", bufs=5))
            bpool = stk.enter_context(tc.tile_pool(name="bp", bufs=2))
            npool = stk.enter_context(tc.tile_pool(name="np_", bufs=2))
            espool = stk.enter_context(tc.tile_pool(name="es", bufs=12))
            x8pool = stk.enter_context(tc.tile_pool(name="x8", bufs=1))
            psm = stk.enter_context(tc.tile_pool(name="psm", bufs=6, space="PSUM"))
            psr = stk.enter_context(tc.tile_pool(name="psr", bufs=2, space="PSUM"))
            dpool = stk.enter_context(tc.tile_pool(name="dram", bufs=2, space="DRAM"))
            # ---- constants (memset cannot write f32r; stage via f32 + copy)
            ones_f32 = cpool.tile([P, 1], F32)
            nc.vector.memset(ones_f32[:], 1.0)
            ones_f16 = cpool.tile([P, 1], F16)
            nc.vector.memset(ones_f16[:], 1.0)
            trimask = cpool.tile([P, P], F16)
            nc.sync.dma_start(trimask[:], masks[:])
            ones_col = cpool.tile([P, 1], F32R)
            nc.vector.tensor_copy(ones_col[:], ones_f32[:])
            eps1 = cpool.tile([1, 1], F32)
            nc.vector.memset(eps1[:], EPS)
            sel4_sb = cpool.tile([P, B], F32R)
            nc.sync.dma_start(sel4_sb[:], sel4[:])
            remw_sb = cpool.tile([P, 1], F32)
            nc.sync.dma_start(remw_sb[:], remw[:])

            # persistent K^T / V buffers
            KTo = [ppool.tile([P, T], F16, tag=f"kto{e}", name=f"KTo{e}")
                   for e in range(DT)]
            KTr = [ppool.tile([P, T], F16, tag=f"ktr{e}", name=f"KTr{e}")
                   for e in range(DT)]
            VT = [ppool.tile([P, VW], F16, tag=f"vt{j}", name=f"VT{j}")
                  for j in range(KB)]
            for j in range(4):
                for h in range(NH):
                    nc.vector.tensor_copy(
                        VT[j][:, h * (DH + 1) + DH : h * (DH + 1) + DH + 1],
                        ones_f32[:])

            # pair rank register for dynamic reads of the AllGather output
            with tc.tile_critical():
                with nc.sync.register("pairreg") as preg:
                    nc.sync.reg_load(preg, pairsel[0:1, 0:1])
                    pv = nc.sync.snap(preg, min_val=0, max_val=NC_ - 1)

            # ---- residual stream (updated in place by residual adds)
            xT = []
            for k in range(DT):
                t_ = xpool.tile([P, T], F32R, tag="xT", name=f"xT{k}")
                nc.sync.dma_start(t_[:], x0T[k])
                xT.append(t_)

            def ln_stats(w1_sb, lname):
                """Fold-LN prep: returns (xh f16 tiles, rstd_b, mrs_b,
                negmean_h row, rstd_row) — no barrier on the PE."""
                sum_ps = psm.tile([1, T], F32, tag="acc", space="PSUM",
                                  name=f"{lname}sum")
                sq_ps = psm.tile([1, T], F32, tag="acc", space="PSUM",
                                 name=f"{lname}sq")
                xh = []
                for k in range(DT):
                    h_ = hpool.tile([P, T], F16, tag="hT", name=f"{lname}xh{k}")
                    nc.scalar.copy(h_[:], xT[k][:])
                    xsq = epool.tile([P, T], F16, tag="lnsq", name="xsq", bufs=2)
                    nc.vector.tensor_mul(xsq[:], h_[:], h_[:])
                    nc.tensor.matmul(sum_ps[:], ones_f16[:], h_[:],
                                     start=(k == 0), stop=(k == DT - 1))
                    nc.tensor.matmul(sq_ps[:], ones_f16[:], xsq[:],
                                     start=(k == 0), stop=(k == DT - 1))
                    xh.append(h_)
                sums_sb = spool.tile([1, T], F32, tag="lnstat", name="sums_sb")
                nc.vector.tensor_copy(sums_sb[:], sum_ps[:])
                m2s = spool.tile([1, T], F32, tag="lnstat", name="m2s")
                nc.vector.scalar_tensor_tensor(out=m2s[:], in0=sums_sb[:],
                                               scalar=1.0 / (D * D), in1=sums_sb[:],
                                               op0=OP.mult, op1=OP.mult)
                var = spool.tile([1, T], F32, tag="lnstat", name="var")
                nc.vector.scalar_tensor_tensor(out=var[:], in0=sq_ps[:],
                                               scalar=1.0 / D, in1=m2s[:],
                                               op0=OP.mult, op1=OP.subtract)
                std = spool.tile([1, T], F32, tag="lnstat", name="std")
                nc.scalar.activation(std[:], var[:], AF.Sqrt, bias=eps1[:])
                rstd = spool.tile([1, T], F32, tag="lnr", name="rstd", bufs=4)
                nc.vector.reciprocal_approx_fast(out=rstd[:], in_=std[:])
                negmean_h = spool.tile([1, T], F16, tag="lnr", name="negmean_h", bufs=4)
                nc.vector.tensor_scalar_mul(negmean_h[:], sums_sb[:], -1.0 / D)
                rstd_b = bpool.tile([P, T], F32, tag="lnb", name="rstd_b")
                nc.gpsimd.partition_broadcast(rstd_b[:], rstd[:])
                return xh, rstd_b, negmean_h, rstd

            for l in range(n_layers):
                with nc.named_scope(f"L{l}"):
                    # per-layer small constants (w1 row sums, f16 rows)
                    w1_sb = spool.tile([1, 2 * D + FF], F16, tag="w1", name="w1_sb", bufs=2)
                    nc.sync.dma_start(w1_sb[:], w1r[l])
                    w1v_sb = spool.tile([1, D], F16, tag="w1v", name="w1v_sb", bufs=2)
                    nc.sync.dma_start(w1v_sb[:], w1v[l])

                    xh, rstd_b, negmean_h, rstd = ln_stats(w1_sb, "ln1")
                    # rstd by-partition for the V write (token-major)
                    rmd = dpool.tile([1, T], F32, tag="rmd", name="rmd")
                    nc.sync.dma_start(rmd[:], rstd[:])
                    rstdT = spool.tile([P, 4], F32, tag="rstdT", name="rstdT")
                    nc.sync.dma_start(
                        rstdT[:],
                        rmd[0:1, :].rearrange("o (m p) -> (o p) m", p=P))

                    # ---- K^T, V first (feeds AllGather early), then Q^T
                    wk_sb = []
                    for k in range(DT):
                        wt = wpool.tile([P, D], F16, tag="w", name="wk_sb")
                        nc.sync.dma_start(wt[:], wkT[l][:, k * D : (k + 1) * D])
                        wk_sb.append(wt)
                    ko8 = []
                    for m in range(DT):
                        k_ps = psr.tile([P, T], F32, tag="rot", space="PSUM",
                                        name="k_ps")
                        for k in range(DT):
                            nc.tensor.matmul(
                                k_ps[:], wk_sb[k][:, m * P : (m + 1) * P],
                                xh[k][:], start=(k == 0), stop=False)
                        nc.tensor.matmul(
                            k_ps[:], w1_sb[0:1, m * P : (m + 1) * P],
                            negmean_h[:], start=False, stop=True)
                        nc.vector.tensor_mul(KTo[m][:], k_ps[:], rstd_b[:])
                        k8 = x8pool.tile([P, T], F8, tag=f"ko8{m}", name="k8")
                        nc.vector.tensor_copy(k8[:], KTo[m][:])
                        ko8.append(k8)

                    wv_sb = []
                    for k in range(DT):
                        wt = wpool.tile([P, D], F16, tag="w", name="wv_sb")
                        nc.sync.dma_start(wt[:], wvT[l][:, k * D : (k + 1) * D])
                        wv_sb.append(wt)
                    vo8 = []
                    for m in range(4):
                        for c in range(2):
                            v_ps = psr.tile([P, 6 * DH], F32, tag="rot",
                                            space="PSUM", name="v_ps")
                            for k in range(DT):
                                nc.tensor.matmul(
                                    v_ps[:], xh[k][:, m * P : (m + 1) * P],
                                    wv_sb[k][:, c * 6 * DH : (c + 1) * 6 * DH],
                                    start=(k == 0), stop=False)
                            # rank-1: -= mean_t * w1v  (negmean x w1v outer)
                            nc.tensor.matmul(
                                v_ps[:], negmean_h[0:1, m * P : (m + 1) * P],
                                w1v_sb[0:1, c * 6 * DH : (c + 1) * 6 * DH],
                                start=False, stop=True)
                            dst = VT[m][:, c * 6 * (DH + 1) : (c + 1) * 6 * (DH + 1)] \
                                .rearrange("p (h e) -> p h e", h=6, e=DH + 1)[:, :, 0:DH]
                            src = v_ps[:].rearrange("p (h e) -> p h e", h=6, e=DH)
                            nc.scalar.activation(dst, src, AF.Identity,
                                                 scale=rstdT[:, m : m + 1])
                        v8 = x8pool.tile([P, VW], F8, tag=f"vo8{m}", name="v8")
                        nc.vector.tensor_copy(v8[:], VT[m][:])
                        vo8.append(v8)

                    # ---- share K^T/V with the pair core (8-way fp8 AllGather)
                    contrib = dpool.tile([P, CONTRIB_W], F8, tag="contrib",
                                         name="contrib")
                    for e in range(DT):
                        nc.sync.dma_start(contrib[:, e * T : (e + 1) * T], ko8[e][:])
                    for m in range(4):
                        nc.sync.dma_start(
                            contrib[:, DT * T + m * VW : DT * T + (m + 1) * VW],
                            vo8[m][:])
                    gout = dpool.tile([NC_, P, CONTRIB_W], F8, tag="gout",
                                      addr_space="Shared", name="gout")
                    nc.gpsimd.collective_compute(
                        "AllGather", OP.bypass,
                        ins=[contrib[:].opt()],
                        outs=[gout[:].opt()],
                        replica_groups=[list(range(NC_))],
                    )

                    # ---- pre-AG-independent work: Q^T + own-slot scores
                    wq_sb = []
                    for k in range(DT):
                        wt = wpool.tile([P, D], F16, tag="w", name="wq_sb")
                        nc.sync.dma_start(wt[:], wqT[l][:, k * D : (k + 1) * D])
                        wq_sb.append(wt)
                    QT = []
                    for m in range(DT):
                        q_ps = psr.tile([P, T], F32, tag="rot", space="PSUM",
                                        name="q_ps")
                        for k in range(DT):
                            nc.tensor.matmul(
                                q_ps[:], wq_sb[k][:, m * P : (m + 1) * P],
                                xh[k][:], start=(k == 0), stop=False)
                        nc.tensor.matmul(
                            q_ps[:], w1_sb[0:1, D + m * P : D + (m + 1) * P],
                            negmean_h[:], start=False, stop=True)
                        qt = qpool.tile([P, T], F16, tag="qt", name="qt")
                        nc.vector.tensor_mul(qt[:], q_ps[:], rstd_b[:])
                        QT.append(qt)

                    # phase A: scores+exp+mask for own slots, all 12 heads
                    # (es saved in SBUF; everything here is AG-independent)
                    es_own = {}
                    for h in range(NH):
                        et, base = h // 2, (h % 2) * DH
                        for j in range(4):
                            c0 = j * P
                            N = T - c0
                            pp = psr if h % 2 == 0 else psm
                            s_ps = pp.tile([P, T], F32,
                                           tag="rot" if h % 2 == 0 else "acc",
                                           space="PSUM", name="s_ps")
                            nc.tensor.matmul(
                                s_ps[:, 0:N],
                                KTo[et][base : base + DH, c0 : c0 + P],
                                QT[et][base : base + DH, c0:T],
                                start=True, stop=True)
                            e_sb = espool.tile([P, N], F16, tag=f"esv{j}",
                                               name="e_sb")
                            nc.scalar.activation(e_sb[:], s_ps[:, 0:N],
                                                 AF.Exp, scale=SCALE)
                            nc.vector.tensor_mul(e_sb[:, 0:P], e_sb[:, 0:P],
                                                 trimask[:])
                            es_own[(h, j)] = e_sb

                    # ---- AG-dependent: unpack pair K/V (fp8 -> f16, V *remw)
                    rsrc = gout[bass.ds(pv, 1)]
                    for e in range(DT):
                        kr8 = x8pool.tile([P, T], F8, tag=f"kr8{e}", name="kr8")
                        nc.sync.dma_start(kr8[:], rsrc[0, :, e * T : (e + 1) * T])
                        nc.vector.tensor_copy(KTr[e][:], kr8[:])
                    for m in range(4):
                        vr8 = x8pool.tile([P, VW], F8, tag=f"vr8{m}", name="vr8")
                        nc.sync.dma_start(
                            vr8[:],
                            rsrc[0, :, DT * T + m * VW : DT * T + (m + 1) * VW])
                        nc.vector.tensor_scalar_mul(VT[4 + m][:], vr8[:],
                                                    remw_sb[:, 0:1])

                    # phase B: attnV own (saved es) + remote slots + normalize
                    wo_sb = []
                    for k in range(DT):
                        wt = wpool.tile([P, D], F16, tag="w", name="wo_sb")
                        nc.sync.dma_start(wt[:], woT[l][:, k * D : (k + 1) * D])
                        wo_sb.append(wt)
                    attnC = [apool.tile([P, T], F16, tag="attnC", name=f"attnC{e}")
                             for e in range(DT)]

                    def wo_partial(g):
                        # wo contribution of head-group g's two attnC tiles;
                        # emitted one hg late so its inputs are ready when
                        # the PE drains down to it
                        for m in range(DT):
                            o_ps = psr.tile([P, T], F32, tag="rot", space="PSUM",
                                            name="o_ps")
                            for kk in range(2):
                                k = 2 * g + kk
                                nc.tensor.matmul(
                                    o_ps[:],
                                    wo_sb[k][:, m * P : (m + 1) * P],
                                    attnC[k][:], start=(kk == 0), stop=(kk == 1))
                            nc.vector.tensor_add(xT[m][:], o_ps[:], xT[m][:])

                    for hg in range(0, NH, 4):
                        attn_ps = {}
                        for h in range(hg, hg + 4):
                            attn_ps[h] = psm.tile([DH + 1, T], F32, tag="acc",
                                                  space="PSUM", name=f"attnps{h}")
                        for j in range(4):
                            c0 = j * P
                            for h in range(hg, hg + 4):
                                nc.tensor.matmul(
                                    attn_ps[h][:, c0:T],
                                    VT[j][:, h * (DH + 1) : (h + 1) * (DH + 1)],
                                    es_own[(h, j)][:],
                                    start=(j == 0), stop=False)
                        for j in range(4, KB):
                            es = {}
                            for h in range(hg, hg + 4):
                                et, base = h // 2, (h % 2) * DH
                                pp = psr if h % 2 == 0 else psm
                                s_ps = pp.tile([P, T], F32,
                                               tag="rot" if h % 2 == 0 else "acc",
                                               space="PSUM", name="s_ps")
                                nc.tensor.matmul(
                                    s_ps[:],
                                    KTr[et][base : base + DH, (j - 4) * P : (j - 3) * P],
                                    QT[et][base : base + DH, :],
                                    start=True, stop=True)
                                e_sb = epool.tile([P, T], F16, tag="e", name="e_sb", bufs=5)
                                nc.scalar.activation(e_sb[:], s_ps[:],
                                                     AF.Exp, scale=SCALE)
                                es[h] = e_sb
                            for h in range(hg, hg + 4):
                                nc.tensor.matmul(
                                    attn_ps[h][:],
                                    VT[j][:, h * (DH + 1) : (h + 1) * (DH + 1)],
                                    es[h][:],
                                    start=False, stop=(j == KB - 1))
                        for h in range(hg, hg + 4):
                            den = spool.tile([1, T], F32, tag="recip", name="den", bufs=3)
                            nc.vector.tensor_copy(den[:], attn_ps[h][DH : DH + 1, :])
                            recip = spool.tile([1, T], F32, tag="recip", name="recip", bufs=3)
                            nc.vector.reciprocal_approx_fast(out=recip[:], in_=den[:])
                            nrm_b = npool.tile([DH, T], F32, tag="nrm", name="nrm_b")
                            nc.gpsimd.partition_broadcast(nrm_b[:], recip[:])
                            base = (h % 2) * DH
                            nc.vector.tensor_mul(
                                attnC[h // 2][base : base + DH, :],
                                attn_ps[h][0:DH, :], nrm_b[:])
                        if hg >= 4:
                            wo_partial(hg // 4 - 1)
                    wo_partial(2)

                    # ---- FFN (fold-LN + native gelu)
                    xh2, rstd2_b, negmean2_h, _r2 = ln_stats(w1_sb, "ln2")
                    x2_ps = [psm.tile([P, T], F32, tag="acc", space="PSUM",
                                      name=f"x2ps{m}") for m in range(DT)]
                    for f in range(FT):
                        f1w = fpool.tile([P, DT * P], F16, tag="f1w", name="f1w")
                        nc.sync.dma_start(f1w[:], fc1T[l, f])
                        f1_ps = psr.tile([P, T], F32, tag="rot", space="PSUM",
                                         name="f1_ps")
                        for k in range(DT):
                            nc.tensor.matmul(f1_ps[:], f1w[:, k * P : (k + 1) * P],
                                             xh2[k][:], start=(k == 0),
                                             stop=False)
                        nc.tensor.matmul(
                            f1_ps[:], w1_sb[0:1, 2 * D + f * P : 2 * D + (f + 1) * P],
                            negmean2_h[:], start=False, stop=True)
                        t2 = epool.tile([P, T], F32, tag="lntmp", name="f1fix", bufs=4)
                        nc.vector.tensor_mul(t2[:], f1_ps[:], rstd2_b[:])
                        g_sb = epool.tile([P, T], F16, tag="e", name="g_sb", bufs=5)
                        nc.scalar.activation(g_sb[:], t2[:], AF.Gelu_apprx_tanh)
                        f2w = fpool.tile([P, D], F16, tag="f2w", name="f2w")
                        nc.sync.dma_start(f2w[:], fc2T[l, f])
                        for m in range(DT):
                            nc.tensor.matmul(x2_ps[m][:], f2w[:, m * P : (m + 1) * P],
                                             g_sb[:], start=(f == 0),
                                             stop=(f == FT - 1))
                    for m in range(DT):
                        nc.vector.tensor_add(xT[m][:], x2_ps[m][:], xT[m][:])

            # ---- final: masked AllReduce of predicted token's x column
            with nc.named_scope("final"):
                if dbg:
                    for k in range(DT):
                        nc.sync.dma_start(dbgx[k], xT[k][:].bitcast(F32))
                cont = dpool.tile([P, DT * B], F32, tag="cont", name="cont")
                csb = spool.tile([P, DT * B], F32, tag="csb", name="csb", bufs=1)
                for k in range(DT):
                    nc.vector.tensor_mul(
                        csb[:, k * B : (k + 1) * B],
                        xT[k][:, pcol : pcol + 1].to_broadcast((P, B)),
                        sel4_sb[:])
                nc.sync.dma_start(cont[:], csb[:])
                ar_out = dpool.tile([P, DT * B], F32, tag="arout",
                                    addr_space="Shared", name="ar_out")
                nc.gpsimd.collective_compute(
                    "AllReduce", OP.add,
                    ins=[cont[:].opt()],
                    outs=[ar_out[:].opt()],
                    replica_groups=[list(range(NC_))],
                )
                xf_raw = spool.tile([P, DT * B], F32, tag="xfraw", name="xf_raw", bufs=1)
                nc.sync.dma_start(xf_raw[:], ar_out[:])
                xf = spool.tile([P, DT * B], F32R, tag="xf", name="xf", bufs=1)
                nc.vector.tensor_copy(xf[:], xf_raw[:])

                fs_ps = psm.tile([1, B], F32, tag="acc", space="PSUM", name="fs_ps")
                fq_ps = psm.tile([1, B], F32, tag="acc", space="PSUM", name="fq_ps")
                xfsq = spool.tile([P, DT * B], F32R, tag="xfsq", name="xfsq", bufs=1)
                nc.vector.tensor_mul(xfsq[:], xf[:], xf[:])
                for k in range(DT):
                    nc.tensor.matmul(fs_ps[:], ones_col[:], xf[:, k * B : (k + 1) * B],
                                     start=(k == 0), stop=(k == DT - 1))
                    nc.tensor.matmul(fq_ps[:], ones_col[:], xfsq[:, k * B : (k + 1) * B],
                                     start=(k == 0), stop=(k == DT - 1))
                fmean = spool.tile([1, B], F32, tag="lnstat", name="fmean")
                nc.vector.tensor_scalar_mul(fmean[:], fs_ps[:], 1.0 / D)
                fm2 = spool.tile([1, B], F32, tag="lnstat", name="fm2")
                nc.vector.tensor_mul(fm2[:], fmean[:], fmean[:])
                fsqd = spool.tile([1, B], F32, tag="lnstat", name="fsqd")
                nc.vector.tensor_scalar_mul(fsqd[:], fq_ps[:], 1.0 / D)
                fvar = spool.tile([1, B], F32, tag="lnstat", name="fvar")
                nc.vector.tensor_sub(fvar[:], fsqd[:], fm2[:])
                fstd = spool.tile([1, B], F32, tag="lnstat", name="fstd")
                nc.scalar.activation(fstd[:], fvar[:], AF.Sqrt, bias=eps1[:])
                frstd = spool.tile([1, B], F32, tag="lnr", name="frstd", bufs=4)
                nc.vector.reciprocal(frstd[:], fstd[:])
                fmrsn = spool.tile([1, B], F32, tag="lnr", name="fmrsn", bufs=4)
                nc.vector.scalar_tensor_tensor(out=fmrsn[:], in0=fmean[:],
                                               scalar=-1.0, in1=frstd[:],
                                               op0=OP.mult, op1=OP.mult)
                # transpose [frstd | -mean*rstd] rows to per-partition columns
                fpack = spool.tile([1, 2 * B], F32, tag="lnstat", name="fpack")
                nc.vector.tensor_copy(fpack[:, 0:B], frstd[:])
                nc.vector.tensor_copy(fpack[:, B : 2 * B], fmrsn[:])
                frd = dpool.tile([1, 2 * B], F32, tag="frd", name="frd")
                nc.sync.dma_start(frd[:], fpack[:])
                frstd_c = spool.tile([B, 1], F32, tag="lnr", name="frstd_c", bufs=4)
                nc.sync.dma_start(frstd_c[:], frd[0:1, 0:B].rearrange("o a -> a o"))
                fmrsn_c = spool.tile([B, 1], F32, tag="lnr", name="fmrsn_c", bufs=4)
                nc.sync.dma_start(fmrsn_c[:],
                                  frd[0:1, B : 2 * B].rearrange("o a -> a o"))
                # unembed on RAW xf (LN folded into a per-batch correction)
                xfn = spool.tile([P, DT * B], F16, tag="xfn", name="xfn", bufs=1)
                nc.scalar.copy(xfn[:], xf[:])
                u1b = spool.tile([B, VS], F16, tag="u1b", name="u1b", bufs=1)
                nc.sync.dma_start(u1b[:], u1d[0:1, :].partition_broadcast(B).opt())

                for ci in range(VCH):
                    lg_ps = psr.tile([B, VCW], F32, tag="rot", space="PSUM",
                                     name="lg_ps")
                    for k in range(DT):
                        u_sb = qpool.tile([P, VCW], F16, tag="qt", name="u_sb")
                        nc.sync.dma_start(u_sb[:], uT[k, :, ci * VCW : (ci + 1) * VCW])
                        nc.tensor.matmul(lg_ps[:], xfn[:, k * B : (k + 1) * B], u_sb[:],
                                         start=(k == 0), stop=(k == DT - 1))
                    tsc = fpool.tile([B, VCW], F32, tag="och", name="tsc", bufs=2)
                    nc.scalar.activation(tsc[:], lg_ps[:], AF.Identity,
                                         scale=frstd_c[:])
                    och = fpool.tile([B, VCW], F32, tag="och", name="och", bufs=2)
                    nc.vector.scalar_tensor_tensor(
                        out=och[:], in0=u1b[:, ci * VCW : (ci + 1) * VCW],
                        scalar=fmrsn_c[:], in1=tsc[:],
                        op0=OP.mult, op1=OP.add)
                    nc.sync.dma_start(out[:, ci * VCW : (ci + 1) * VCW], och[:])

    nc.compile()
    return nc


# ---------------------------------------------------------------- host side
def _positional_encoding(s, d):
    idx = np.arange(d)
    exponent = ((2 * (idx // 2)).astype(np.float32) / float(d)).astype(np.float32)
    pos = np.arange(s, dtype=np.float32)[:, None]
    angle = pos / np.power(np.float32(10000.0), exponent[None, :], dtype=np.float32)
    return np.where((idx % 2 == 0)[None, :], np.sin(angle), np.cos(angle)).astype(np.float32)


def _build_masks():
    """trimask[r, c] = 1 if key r <= query c (within-block causal)."""
    r = np.arange(P)
    return (r[:, None] <= r[None, :]).astype(np.float16)


def prepare_inputs(tokens, predict_idx, embedding, ln1_g, ln1_b, wq, wk, wv, wo,
                   ln2_g, ln2_b, fc1, fc2, lnf_g, lnf_b, unembed, n_layers=NL):
    f = lambda a: np.ascontiguousarray(np.asarray(a), dtype=np.float32)
    # the fold-LN kernel exploits gamma=1 / beta=0 (true for this model)
    for g in (ln1_g, ln2_g, lnf_g):
        assert np.allclose(np.asarray(g), 1.0), "LN gamma must be 1"
    for b in (ln1_b, ln2_b, lnf_b):
        assert np.allclose(np.asarray(b), 0.0), "LN beta must be 0"
    tokens = np.asarray(tokens)
    emb = f(embedding)
    pos = _positional_encoding(S, D)

    def wlayout(a):  # [L, out, in] -> [L, P, DT*D] with [l, p, k*D + dout]
        aT = a.transpose(0, 2, 1)
        return np.ascontiguousarray(
            aT.reshape(n_layers, DT, P, D).transpose(0, 2, 1, 3)
            .reshape(n_layers, P, DT * D)).astype(np.float16)

    wqf = f(wq)[:n_layers].reshape(-1, NH * DH, D)
    wkf = f(wk)[:n_layers].reshape(-1, NH * DH, D)
    wvf = f(wv)[:n_layers].reshape(-1, NH * DH, D)
    wqT = wlayout(wqf)
    wkT = wlayout(wkf)
    wvT = wlayout(wvf)
    woT = wlayout(f(wo)[:n_layers])
    fc1f = f(fc1)[:n_layers]
    fc1T = np.ascontiguousarray(
        fc1f.transpose(0, 2, 1)
        .reshape(n_layers, DT, P, FT, P).transpose(0, 3, 2, 1, 4)
        .reshape(n_layers, FT, P, DT * P)).astype(np.float16)
    fc2T = np.ascontiguousarray(
        f(fc2)[:n_layers].transpose(0, 2, 1)
        .reshape(n_layers, FT, P, D)).astype(np.float16)
    uTf = np.ascontiguousarray(f(unembed).T.reshape(DT, P, V)).astype(np.float16)
    u1f = np.ascontiguousarray(
        f(unembed).astype(np.float16).astype(np.float32).sum(-1)
        .reshape(1, V)).astype(np.float16)

    # packed row sums [L, 1, 768(K) + 768(Q) + 3072(fc1)] for the rank-1
    # -mean*w1 correction matmuls (rhs = negmean row)
    w1r_ = np.ascontiguousarray(np.concatenate(
        [wkf.sum(-1), wqf.sum(-1), fc1f.sum(-1)],
        axis=1).reshape(n_layers, 1, 2 * D + FF)).astype(np.float16)
    w1v_ = np.ascontiguousarray(wvf.sum(-1).reshape(n_layers, 1, D)).astype(np.float16)

    masks = _build_masks()
    pidx = int(predict_idx)
    in_maps = []
    for c in range(NC_):
        b, h = c // 2, c % 2
        toks = np.asarray(tokens[b, h * T : (h + 1) * T]).astype(np.int64)
        x0 = emb.T[toks] + pos[h * T : (h + 1) * T]
        x0T = np.ascontiguousarray(x0.T.reshape(DT, P, T)).astype(np.float32)
        sel4 = np.zeros((P, B), np.float32)
        if pidx // T == h:
            sel4[:, b] = 1.0
        m = {
            "x0T": x0T, "wqT": wqT, "wkT": wkT, "wvT": wvT, "woT": woT,
            "fc1T": fc1T, "fc2T": fc2T, "w1r": w1r_, "w1v": w1v_,
            "uT": uTf[:, :, c * VS : (c + 1) * VS].copy(),
            "u1d": u1f[:, c * VS : (c + 1) * VS].copy(),
            "masks": masks,
            "remw": np.full((P, 1), 1.0 if h == 1 else 0.0, np.float32),
            "sel4": sel4,
            "pairsel": np.array([[c ^ 1]], np.int32),
        }
        in_maps.append(m)
    return in_maps


_CACHED = {}


def kernel(**inputs):
    from concourse.bass_utils import run_bass_kernel_spmd
    pidx = int(np.asarray(inputs["predict_idx"]))
    key = ("nc", pidx % T)
    if key not in _CACHED:
        _CACHED[key] = build_nc(pcol=pidx % T)
    nc = _CACHED[key]
    in_maps = prepare_inputs(**inputs)
    res = run_bass_kernel_spmd(nc, in_maps, core_ids=list(range(NC_)), trace=False)
    return np.concatenate([res.results[c]["out"] for c in range(NC_)], axis=1)
